# revision 1
# baseline (speedup 1.0000x reference)
"""GCN2Net (GCNII) forward pass on 8 Trainium2 NeuronCores.

Strategy (graph/data parallel, per sharding hint):
  - Destination nodes are partitioned across the 8 cores (12500 each).
  - Node features live in an AllGathered fp16 table in HBM (the "halo
    exchange": every layer each core contributes its slab of
    g = dinv * h, and reads arbitrary rows during message passing).
  - Message passing per layer: edges (sorted by destination block) are
    processed in 128-edge tiles.  Source rows are fetched with batched
    indirect DMA (gather) from the table; a one-hot selection matrix S
    ([128 edges x 128 dests], built on the vector engine by comparing the
    per-edge local-destination id against an iota row) turns the
    segment-sum into a TensorEngine matmul accumulated in PSUM:
        agg_block = S^T @ G.
  - Symmetric normalization is factorized: deg^-1/2 A deg^-1/2 h
    = dinv * (A_raw @ (dinv * h)), so S stays exactly 0/1 and the dinv
    scales fold into per-partition scalar ops.
  - GCNII update: s = (1-a)*agg + a*h0;  h = relu(s @ ((1-b)I + b*W))
    with the ((1-b)I + b*W) mix folded on the host into one matrix.
"""

import math
import sys

sys.path.insert(0, "/opt/trn_rl_repo")

import numpy as np

# ----- problem constants (hardcoded per spec) -----
N = 100000
E = 3200000
D = 128
LAYERS = 8
ALPHA = 0.1
THETA = 0.5
NCORES = 8

GB = 64   # gather batch, in 128-edge tiles
SG = 8    # S-build group, in tiles

_BUILD_CACHE = {}
LAST_RESULTS = None  # BassKernelResults of the most recent run (for profiling)


def _host_prepare(x, W_in, b_in, W_gcn, W_out, b_out, edge_index,
                  n, ncores, layers, alpha, theta):
    """Graph preprocessing + per-core input arrays."""
    d = x.shape[1]
    npc = n // ncores
    nblk = math.ceil(npc / 128)
    npc_pad = nblk * 128

    row = np.asarray(edge_index[0], dtype=np.int64)
    col = np.asarray(edge_index[1], dtype=np.int64)
    sl = np.arange(n, dtype=np.int64)
    row = np.concatenate([row, sl])
    col = np.concatenate([col, sl])

    deg = np.bincount(col, minlength=n).astype(np.float64)
    dinv = (1.0 / np.sqrt(deg)).astype(np.float32)  # deg >= 1 (self loops)

    # table row of each source node (tables are [ncores*npc_pad, d])
    tbl = ((row // npc) * npc_pad + (row % npc)).astype(np.int32)
    dstcore = (col // npc).astype(np.int64)
    dstlocal = col % npc
    blk = (dstlocal // 128).astype(np.int64)
    dloc = (dstlocal % 128).astype(np.int64)

    order = np.lexsort((blk, dstcore))
    tbl_o = tbl[order]
    blk_o = blk[order]
    dloc_o = dloc[order]
    key_o = dstcore[order] * nblk + blk_o

    ecnt = np.zeros((ncores, nblk), dtype=np.int64)
    np.add.at(ecnt, (dstcore, blk), 1)
    tiles_b = np.maximum(1, np.ceil(ecnt.max(axis=0) / 128).astype(np.int64))
    ttot = int(tiles_b.sum())
    tstart = np.concatenate([[0], np.cumsum(tiles_b)])[:-1].astype(np.int64)

    idx_all = np.zeros((ncores, 128, ttot), np.int32)
    dst_all = np.full((ncores, 128, ttot), -1.0, np.float16)
    bounds = np.searchsorted(key_o, np.arange(ncores * nblk + 1))
    for c in range(ncores):
        for b in range(nblk):
            s, e = bounds[c * nblk + b], bounds[c * nblk + b + 1]
            m = e - s
            if m == 0:
                continue
            t0 = tstart[b]
            part = np.arange(m) % 128
            t = np.arange(m) // 128
            idx_all[c, part, t0 + t] = tbl_o[s:e]
            dst_all[c, part, t0 + t] = dloc_o[s:e].astype(np.float16)

    # per-layer weight mix (1-beta) I + beta W
    wt_all = np.zeros((d, layers * d), np.float32)
    for i in range(layers):
        beta = float(np.log(theta / (i + 1) + 1.0))
        wt_all[:, i * d:(i + 1) * d] = (
            (1.0 - beta) * np.eye(d, dtype=np.float32)
            + beta * np.asarray(W_gcn[i], dtype=np.float32)
        )

    iota = np.tile(np.arange(128, dtype=np.float16), SG)[None, :].repeat(128, 0)

    in_maps = []
    for c in range(ncores):
        nd = np.zeros((128, nblk), np.float32)
        loc = np.arange(npc)
        nd[loc % 128, loc // 128] = dinv[c * npc:(c + 1) * npc]
        xt = np.zeros((d, npc_pad), np.float32)
        xt[:, :npc] = np.asarray(x[c * npc:(c + 1) * npc], np.float32).T
        in_maps.append({
            "xT": np.ascontiguousarray(xt),
            "idx": np.ascontiguousarray(idx_all[c]),
            "dst": np.ascontiguousarray(dst_all[c]),
            "iota": iota.astype(np.float16),
            "dinvp": nd,
            "dinv09": (1.0 - alpha) * nd,
            "Wt": wt_all,
            "Win": np.asarray(W_in, np.float32),
            "binrow": np.asarray(b_in, np.float32)[None, :],
            "Wout": np.asarray(W_out, np.float32),
            "bout": np.full((128, 1), float(np.asarray(b_out).ravel()[0]), np.float32),
        })
    meta = dict(n=n, d=d, ncores=ncores, layers=layers, npc=npc, nblk=nblk,
                npc_pad=npc_pad, tiles_b=tuple(int(t) for t in tiles_b),
                ttot=ttot, alpha=alpha)
    return in_maps, meta


def _build_program(meta):
    import concourse.bacc as bacc
    import concourse.bass as bass
    import concourse.mybir as mybir
    import concourse.tile as tile
    from concourse.masks import make_identity

    d = meta["d"]
    ncores = meta["ncores"]
    layers = meta["layers"]
    nblk = meta["nblk"]
    npc = meta["npc"]
    npc_pad = meta["npc_pad"]
    tiles_b = meta["tiles_b"]
    ttot = meta["ttot"]
    ntab = ncores * npc_pad
    f32 = mybir.dt.float32
    f16 = mybir.dt.float16
    rg = [list(range(ncores))]

    nc = bacc.Bacc("TRN2", target_bir_lowering=False, debug=False,
                   num_devices=ncores)

    xT_t = nc.dram_tensor("xT", [d, npc_pad], f32, kind="ExternalInput")
    idx_t = nc.dram_tensor("idx", [128, ttot], mybir.dt.int32, kind="ExternalInput")
    dst_t = nc.dram_tensor("dst", [128, ttot], f16, kind="ExternalInput")
    iota_t = nc.dram_tensor("iota", [128, SG * 128], f16, kind="ExternalInput")
    dinvp_t = nc.dram_tensor("dinvp", [128, nblk], f32, kind="ExternalInput")
    dinv09_t = nc.dram_tensor("dinv09", [128, nblk], f32, kind="ExternalInput")
    wt_t = nc.dram_tensor("Wt", [d, layers * d], f32, kind="ExternalInput")
    win_t = nc.dram_tensor("Win", [d, d], f32, kind="ExternalInput")
    binrow_t = nc.dram_tensor("binrow", [1, d], f32, kind="ExternalInput")
    wout_t = nc.dram_tensor("Wout", [d, 1], f32, kind="ExternalInput")
    bout_t = nc.dram_tensor("bout", [128, 1], f32, kind="ExternalInput")
    yout_t = nc.dram_tensor("yout", [npc_pad, 1], f32, kind="ExternalOutput")

    ag_in_t = nc.dram_tensor("ag_in", [npc_pad, d], f16)
    gtabs = [
        nc.dram_tensor(f"gtab{p}", [ntab, d], f16, addr_space="Shared")
        for p in range(2)
    ]

    # global tile stream -> block id
    blk_of = []
    for b in range(nblk):
        blk_of += [b] * tiles_b[b]
    tstart = np.concatenate([[0], np.cumsum(tiles_b)])[:-1]
    tend = np.cumsum(tiles_b)

    with tile.TileContext(nc) as tc:
        with (
            tc.tile_pool(name="const", bufs=1) as constp,
            tc.tile_pool(name="gbatch", bufs=2) as gbp,
            tc.tile_pool(name="sgrp", bufs=4) as sgp,
            tc.tile_pool(name="blk", bufs=3) as blkp,
            tc.tile_pool(name="pagg", bufs=2, space="PSUM") as paggp,
            tc.tile_pool(name="ptp", bufs=2, space="PSUM") as ptpp,
            tc.tile_pool(name="pz", bufs=2, space="PSUM") as pzp,
        ):
            idx_sb = constp.tile([128, ttot], mybir.dt.int32)
            nc.sync.dma_start(out=idx_sb[:], in_=idx_t[:])
            dst_sb = constp.tile([128, ttot], f16)
            nc.sync.dma_start(out=dst_sb[:], in_=dst_t[:])
            iota_sb = constp.tile([128, SG * 128], f16)
            nc.sync.dma_start(out=iota_sb[:], in_=iota_t[:])
            dinvp_sb = constp.tile([128, nblk], f32)
            nc.sync.dma_start(out=dinvp_sb[:], in_=dinvp_t[:])
            dinv09_sb = constp.tile([128, nblk], f32)
            nc.sync.dma_start(out=dinv09_sb[:], in_=dinv09_t[:])
            wt_sb = constp.tile([128, layers * d], f32)
            nc.sync.dma_start(out=wt_sb[:d], in_=wt_t[:])
            win_sb = constp.tile([128, d], f32)
            nc.sync.dma_start(out=win_sb[:d], in_=win_t[:])
            binrow_sb = constp.tile([128, d], f32)
            nc.sync.dma_start(out=binrow_sb[:1], in_=binrow_t[:])
            wout_sb = constp.tile([128, 1], f32)
            nc.sync.dma_start(out=wout_sb[:d], in_=wout_t[:])
            bout_sb = constp.tile([128, 1], f32)
            nc.sync.dma_start(out=bout_sb[:], in_=bout_t[:])
            ones_sb = constp.tile([128, 128], f32)
            nc.vector.memset(ones_sb[:], 1.0)
            ident_sb = constp.tile([128, 128], f32)
            make_identity(nc, ident_sb[:])
            h0s_sb = constp.tile([128, npc_pad], f32)

            # ---- prologue: h0 = x @ W_in + b_in ----
            for b in range(nblk):
                xt = blkp.tile([128, 128], f32, tag="xt")
                nc.sync.dma_start(out=xt[:d], in_=xT_t[:, b * 128:(b + 1) * 128])
                ph = pzp.tile([128, 128], f32, tag="z")
                nc.tensor.matmul(ph[:], lhsT=xt[:d], rhs=win_sb[:d],
                                 start=True, stop=False)
                nc.tensor.matmul(ph[:], lhsT=ones_sb[:1], rhs=binrow_sb[:1],
                                 start=False, stop=True)
                nc.vector.tensor_scalar(
                    out=h0s_sb[:, b * 128:(b + 1) * 128], in0=ph[:],
                    scalar1=ALPHA, scalar2=None, op0=mybir.AluOpType.mult)
                g0 = blkp.tile([128, 128], f16, tag="gout")
                nc.scalar.activation(out=g0[:], in_=ph[:],
                                     func=mybir.ActivationFunctionType.Copy,
                                     scale=dinvp_sb[:, b:b + 1])
                nc.sync.dma_start(out=ag_in_t[b * 128:(b + 1) * 128, :], in_=g0[:])
            nc.gpsimd.collective_compute(
                "AllGather", mybir.AluOpType.bypass, replica_groups=rg,
                ins=[ag_in_t[:]], outs=[gtabs[0][:]])

            # ---- layers ----
            for i in range(layers):
                src_tab = gtabs[i % 2]
                dst_tab = gtabs[(i + 1) % 2]
                last = i == layers - 1
                gt = None
                st = None
                for j in range(ttot):
                    b = blk_of[j]
                    if j % GB == 0:
                        bt = min(GB, ttot - j)
                        gt = gbp.tile([128, GB, 128], f16, tag="g")
                        nc.gpsimd.indirect_dma_start(
                            out=gt[:, :bt, :], out_offset=None,
                            in_=src_tab[:],
                            in_offset=bass.IndirectOffsetOnAxis(
                                ap=idx_sb[:, j:j + bt], axis=0))
                    if j % SG == 0:
                        sgt = min(SG, ttot - j)
                        st = sgp.tile([128, SG, 128], f16, tag="s")
                        nc.vector.tensor_tensor(
                            out=st[:, :sgt, :],
                            in0=dst_sb[:, j:j + sgt][:, :, None]
                                .to_broadcast([128, sgt, 128]),
                            in1=iota_sb[:].rearrange("p (g e) -> p g e", e=128)[:, :sgt, :],
                            op=mybir.AluOpType.is_equal)
                    if j == tstart[b]:
                        agg = paggp.tile([128, 128], f32, tag="agg",
                                         name=f"agg_{i}_{b}")
                    nc.tensor.matmul(
                        agg[:], lhsT=st[:, j % SG, :], rhs=gt[:, j % GB, :],
                        start=(j == tstart[b]), stop=(j == tend[b] - 1))
                    if j == tend[b] - 1:
                        # close block b
                        s_sb = blkp.tile([128, 128], f32, tag="s_sb")
                        nc.vector.tensor_scalar(
                            out=s_sb[:], in0=agg[:],
                            scalar1=dinv09_sb[:, b:b + 1], scalar2=None,
                            op0=mybir.AluOpType.mult)
                        nc.vector.tensor_tensor(
                            out=s_sb[:], in0=s_sb[:],
                            in1=h0s_sb[:, b * 128:(b + 1) * 128],
                            op=mybir.AluOpType.add)
                        stp = ptpp.tile([128, 128], f32, tag="tp")
                        nc.tensor.transpose(out=stp[:], in_=s_sb[:],
                                            identity=ident_sb[:])
                        st_sb = blkp.tile([128, 128], f32, tag="st_sb")
                        nc.scalar.copy(out=st_sb[:], in_=stp[:])
                        z = pzp.tile([128, 128], f32, tag="z")
                        nc.tensor.matmul(z[:], lhsT=st_sb[:],
                                         rhs=wt_sb[:d, i * d:(i + 1) * d],
                                         start=True, stop=True)
                        if not last:
                            go = blkp.tile([128, 128], f16, tag="gout")
                            nc.scalar.activation(
                                out=go[:], in_=z[:],
                                func=mybir.ActivationFunctionType.Relu,
                                scale=dinvp_sb[:, b:b + 1])
                            nc.sync.dma_start(
                                out=ag_in_t[b * 128:(b + 1) * 128, :], in_=go[:])
                        else:
                            h_sb = blkp.tile([128, 128], f32, tag="s_sb")
                            nc.scalar.activation(
                                out=h_sb[:], in_=z[:],
                                func=mybir.ActivationFunctionType.Relu)
                            htp = ptpp.tile([128, 128], f32, tag="tp")
                            nc.tensor.transpose(out=htp[:], in_=h_sb[:],
                                                identity=ident_sb[:])
                            ht_sb = blkp.tile([128, 128], f32, tag="st_sb")
                            nc.scalar.copy(out=ht_sb[:], in_=htp[:])
                            op = pzp.tile([128, 128], f32, tag="z")
                            nc.tensor.matmul(op[:, :1], lhsT=ht_sb[:d],
                                             rhs=wout_sb[:d], start=True,
                                             stop=True)
                            o_sb = blkp.tile([128, 1], f32, tag="o_sb")
                            nc.vector.tensor_scalar(
                                out=o_sb[:], in0=op[:, :1],
                                scalar1=bout_sb[:, :1], scalar2=None,
                                op0=mybir.AluOpType.add)
                            nv = min(128, npc - b * 128)
                            nc.sync.dma_start(
                                out=yout_t[b * 128:b * 128 + nv, :],
                                in_=o_sb[:nv])
                if not last:
                    nc.gpsimd.collective_compute(
                        "AllGather", mybir.AluOpType.bypass, replica_groups=rg,
                        ins=[ag_in_t[:]], outs=[dst_tab[:]])

    nc.finalize()
    return nc


def _get_program(meta):
    key = (meta["n"], meta["d"], meta["ncores"], meta["layers"],
           meta["tiles_b"])
    if key not in _BUILD_CACHE:
        _BUILD_CACHE[key] = _build_program(meta)
    return _BUILD_CACHE[key]


def kernel(x, edge_weight, W_in, b_in, W_gcn, W_out, b_out, edge_index,
           trace=False):
    global LAST_RESULTS
    from concourse.bass_utils import run_bass_kernel_spmd

    x = np.asarray(x)
    in_maps, meta = _host_prepare(
        x, W_in, b_in, W_gcn, W_out, b_out, np.asarray(edge_index),
        N, NCORES, LAYERS, ALPHA, THETA)
    nc = _get_program(meta)
    if trace:
        try:
            from axon_profile_shim import install as _shim
            _shim()
        except Exception:
            pass
    res = run_bass_kernel_spmd(nc, in_maps, core_ids=list(range(NCORES)),
                               trace=trace)
    LAST_RESULTS = res
    npc = meta["npc"]
    y = np.empty((N, 1), np.float32)
    for c in range(NCORES):
        y[c * npc:(c + 1) * npc] = res.results[c]["yout"][:npc]
    return y


# revision 3
# speedup vs baseline: 1.6164x; 1.6164x over previous
"""GCN2Net (GCNII) forward pass on 8 Trainium2 NeuronCores.

Strategy (graph/data parallel, per the sharding hint):
  - Destination nodes are partitioned across the 8 cores (12500 each).
  - Node features live in an AllGathered fp16 table in HBM (the "halo
    exchange": every layer each core contributes its slab of
    g = dinv * h and reads arbitrary rows during message passing).
  - Message passing per layer: edges (sorted by destination block) are
    processed in 128-edge tiles.  Each tile's 128 source rows are
    fetched with one indirect DMA (one index per partition — the only
    indexed-gather form this hardware supports); a one-hot selection
    matrix S ([128 edges x 128 dests], built on the vector engine by
    comparing per-edge local-destination ids against an iota row) turns
    the segment-sum into a TensorEngine matmul accumulated in PSUM:
        agg_block = S^T @ G.
  - Symmetric normalization is factorized: deg^-1/2 A deg^-1/2 h
    = dinv * (A_raw @ (dinv * h)), so S stays exactly 0/1 and the dinv
    scales fold into per-partition scalar ops.
  - GCNII update: s = (1-a)*agg + a*h0;  h = relu(s @ ((1-b)I + b*W))
    with ((1-b)I + b*W) folded on the host into one matrix per layer.
"""

import math
import sys

sys.path.insert(0, "/opt/trn_rl_repo")

import numpy as np

# ----- problem constants (hardcoded per spec) -----
N = 100000
E = 3200000
D = 128
LAYERS = 8
ALPHA = 0.1
THETA = 0.5
NCORES = 8

SG = 8     # S-build group, in tiles

_BUILD_CACHE = {}
LAST_RESULTS = None  # BassKernelResults of the most recent run


def _host_prepare(x, W_in, b_in, W_gcn, W_out, b_out, edge_index,
                  n, ncores, layers, alpha, theta):
    d = x.shape[1]
    npc = n // ncores
    nblk = math.ceil(npc / 128)
    npc_pad = nblk * 128

    row = np.asarray(edge_index[0], dtype=np.int64)
    col = np.asarray(edge_index[1], dtype=np.int64)
    sl = np.arange(n, dtype=np.int64)
    row = np.concatenate([row, sl])
    col = np.concatenate([col, sl])

    deg = np.bincount(col, minlength=n).astype(np.float64)
    dinv = (1.0 / np.sqrt(deg)).astype(np.float32)  # deg >= 1 (self loops)

    # table row of each source node (tables are [ncores*npc_pad, d])
    tbl = ((row // npc) * npc_pad + (row % npc)).astype(np.int32)
    dstcore = (col // npc).astype(np.int64)
    dstlocal = col % npc
    blk = (dstlocal // 128).astype(np.int64)
    dloc = (dstlocal % 128).astype(np.int64)

    key = dstcore * nblk + blk
    order = np.argsort(key, kind="stable")
    tbl_o = tbl[order]
    dloc_o = dloc[order]

    cnt = np.zeros(ncores * nblk, dtype=np.int64)
    np.add.at(cnt, key, 1)
    cnt = cnt.reshape(ncores, nblk)
    tiles_b = np.maximum(1, np.ceil(cnt.max(axis=0) / 128).astype(np.int64))
    ttot = int(tiles_b.sum())
    tstart = np.concatenate([[0], np.cumsum(tiles_b)])[:-1].astype(np.int64)

    idx_all = np.zeros((ncores, 128, ttot), np.int32)
    dst_all = np.full((ncores, 128, ttot), -1.0, np.float16)
    bounds = np.searchsorted(key[order], np.arange(ncores * nblk + 1))
    for c in range(ncores):
        for b in range(nblk):
            k = c * nblk + b
            s, e = bounds[k], bounds[k + 1]
            m = e - s
            if m == 0:
                continue
            t0 = tstart[b]
            part = np.arange(m) % 128
            tt = np.arange(m) // 128
            idx_all[c, part, t0 + tt] = tbl_o[s:e]
            dst_all[c, part, t0 + tt] = dloc_o[s:e].astype(np.float16)

    # per-layer weight mix (1-beta) I + beta W
    wt_all = np.zeros((d, layers * d), np.float32)
    for i in range(layers):
        beta = float(np.log(theta / (i + 1) + 1.0))
        wt_all[:, i * d:(i + 1) * d] = (
            (1.0 - beta) * np.eye(d, dtype=np.float32)
            + beta * np.asarray(W_gcn[i], dtype=np.float32)
        )

    iota = np.tile(np.arange(128, dtype=np.float16), SG)[None, :].repeat(128, 0)

    in_maps = []
    for c in range(ncores):
        nd = np.zeros((128, nblk), np.float32)
        loc = np.arange(npc)
        nd[loc % 128, loc // 128] = dinv[c * npc:(c + 1) * npc]
        xt = np.zeros((d, npc_pad), np.float32)
        xt[:, :npc] = np.asarray(x[c * npc:(c + 1) * npc], np.float32).T
        in_maps.append({
            "xT": np.ascontiguousarray(xt),
            "idx": np.ascontiguousarray(idx_all[c]),
            "dst": np.ascontiguousarray(dst_all[c]),
            "iota": iota.astype(np.float16),
            "dinvp": nd,
            "dinv09": (1.0 - alpha) * nd,
            "Wt": wt_all,
            "Win": np.asarray(W_in, np.float32),
            "binrow": np.asarray(b_in, np.float32)[None, :],
            "Wout": np.asarray(W_out, np.float32),
            "bout": np.full((128, 1), float(np.asarray(b_out).ravel()[0]),
                            np.float32),
        })
    meta = dict(n=n, d=d, ncores=ncores, layers=layers, npc=npc, nblk=nblk,
                npc_pad=npc_pad, tiles_b=tuple(int(t) for t in tiles_b),
                ttot=ttot, alpha=alpha)
    return in_maps, meta


def _build_program(meta):
    import concourse.bacc as bacc
    import concourse.bass as bass
    import concourse.mybir as mybir
    import concourse.tile as tile
    from concourse.masks import make_identity

    d = meta["d"]
    ncores = meta["ncores"]
    layers = meta["layers"]
    nblk = meta["nblk"]
    npc = meta["npc"]
    npc_pad = meta["npc_pad"]
    tiles_b = meta["tiles_b"]
    ttot = meta["ttot"]
    ntab = ncores * npc_pad
    f32 = mybir.dt.float32
    f16 = mybir.dt.float16
    rg = [list(range(ncores))]

    tstart = np.concatenate([[0], np.cumsum(tiles_b)])[:-1]
    tend = np.cumsum(tiles_b)
    blk_of = []
    for b in range(nblk):
        blk_of += [b] * tiles_b[b]

    nc = bacc.Bacc("TRN2", target_bir_lowering=False, debug=False,
                   num_devices=ncores)

    xT_t = nc.dram_tensor("xT", [d, npc_pad], f32, kind="ExternalInput")
    idx_t = nc.dram_tensor("idx", [128, ttot], mybir.dt.int32,
                           kind="ExternalInput")
    dst_t = nc.dram_tensor("dst", [128, ttot], f16, kind="ExternalInput")
    iota_t = nc.dram_tensor("iota", [128, SG * 128], f16, kind="ExternalInput")
    dinvp_t = nc.dram_tensor("dinvp", [128, nblk], f32, kind="ExternalInput")
    dinv09_t = nc.dram_tensor("dinv09", [128, nblk], f32, kind="ExternalInput")
    wt_t = nc.dram_tensor("Wt", [d, layers * d], f32, kind="ExternalInput")
    win_t = nc.dram_tensor("Win", [d, d], f32, kind="ExternalInput")
    binrow_t = nc.dram_tensor("binrow", [1, d], f32, kind="ExternalInput")
    wout_t = nc.dram_tensor("Wout", [d, 1], f32, kind="ExternalInput")
    bout_t = nc.dram_tensor("bout", [128, 1], f32, kind="ExternalInput")
    yout_t = nc.dram_tensor("yout", [npc_pad, 1], f32, kind="ExternalOutput")

    ag_in_t = nc.dram_tensor("ag_in", [npc_pad, d], f16)
    gtabs = [
        nc.dram_tensor(f"gtab{p}", [ntab, d], f16, addr_space="Shared")
        for p in range(2)
    ]

    with tile.TileContext(nc) as tc:
        with (
            tc.tile_pool(name="const", bufs=1) as constp,
            tc.tile_pool(name="gt", bufs=24) as gbp,
            tc.tile_pool(name="sgrp", bufs=4) as sgp,
            tc.tile_pool(name="blk", bufs=3) as blkp,
            tc.tile_pool(name="pagg", bufs=2, space="PSUM") as paggp,
            tc.tile_pool(name="ptp", bufs=2, space="PSUM") as ptpp,
            tc.tile_pool(name="pz", bufs=2, space="PSUM") as pzp,
        ):
            idx_sb = constp.tile([128, ttot], mybir.dt.int32)
            nc.sync.dma_start(out=idx_sb[:], in_=idx_t[:])
            dst_sb = constp.tile([128, ttot], f16)
            nc.sync.dma_start(out=dst_sb[:], in_=dst_t[:])
            iota_sb = constp.tile([128, SG * 128], f16)
            nc.sync.dma_start(out=iota_sb[:], in_=iota_t[:])
            dinvp_sb = constp.tile([128, nblk], f32)
            nc.sync.dma_start(out=dinvp_sb[:], in_=dinvp_t[:])
            dinv09_sb = constp.tile([128, nblk], f32)
            nc.sync.dma_start(out=dinv09_sb[:], in_=dinv09_t[:])
            wt_sb = constp.tile([128, layers * d], f32)
            nc.sync.dma_start(out=wt_sb[:d], in_=wt_t[:])
            win_sb = constp.tile([128, d], f32)
            nc.sync.dma_start(out=win_sb[:d], in_=win_t[:])
            binrow_sb = constp.tile([128, d], f32)
            nc.sync.dma_start(out=binrow_sb[:1], in_=binrow_t[:])
            wout_sb = constp.tile([128, 1], f32)
            nc.sync.dma_start(out=wout_sb[:d], in_=wout_t[:])
            bout_sb = constp.tile([128, 1], f32)
            nc.sync.dma_start(out=bout_sb[:], in_=bout_t[:])
            ones_sb = constp.tile([128, 128], f32)
            nc.vector.memset(ones_sb[:], 1.0)
            ident_sb = constp.tile([128, 128], f32)
            make_identity(nc, ident_sb[:])
            h0s_sb = constp.tile([128, npc_pad], f32)

            # ---- prologue: h0 = x @ W_in + b_in ----
            for b in range(nblk):
                xt = blkp.tile([128, 128], f32, tag="xt")
                nc.sync.dma_start(out=xt[:d], in_=xT_t[:, b * 128:(b + 1) * 128])
                ph = pzp.tile([128, 128], f32, tag="z")
                nc.tensor.matmul(ph[:], lhsT=xt[:d], rhs=win_sb[:d],
                                 start=True, stop=False)
                nc.tensor.matmul(ph[:], lhsT=ones_sb[:1], rhs=binrow_sb[:1],
                                 start=False, stop=True)
                nc.vector.tensor_scalar(
                    out=h0s_sb[:, b * 128:(b + 1) * 128], in0=ph[:],
                    scalar1=ALPHA, scalar2=None, op0=mybir.AluOpType.mult)
                g0 = blkp.tile([128, 128], f16, tag="gout")
                nc.scalar.activation(out=g0[:], in_=ph[:],
                                     func=mybir.ActivationFunctionType.Copy,
                                     scale=dinvp_sb[:, b:b + 1])
                nc.sync.dma_start(out=ag_in_t[b * 128:(b + 1) * 128, :], in_=g0[:])
            nc.gpsimd.collective_compute(
                "AllGather", mybir.AluOpType.bypass, replica_groups=rg,
                ins=[ag_in_t[:]], outs=[gtabs[0][:]])

            # ---- layers ----
            for i in range(layers):
                src_tab = gtabs[i % 2]
                dst_tab = gtabs[(i + 1) % 2]
                last = i == layers - 1
                st = None
                for j in range(ttot):
                    b = blk_of[j]
                    gt = gbp.tile([128, 128], f16, tag="g")
                    nc.gpsimd.indirect_dma_start(
                        out=gt[:], out_offset=None,
                        in_=src_tab[:],
                        in_offset=bass.IndirectOffsetOnAxis(
                            ap=idx_sb[:, j:j + 1], axis=0))
                    if j % SG == 0:
                        sgt = min(SG, ttot - j)
                        st = sgp.tile([128, SG, 128], f16, tag="s")
                        nc.vector.tensor_tensor(
                            out=st[:, :sgt, :],
                            in0=dst_sb[:, j:j + sgt][:, :, None]
                                .to_broadcast([128, sgt, 128]),
                            in1=iota_sb[:].rearrange("p (g e) -> p g e", e=128)
                                [:, :sgt, :],
                            op=mybir.AluOpType.is_equal)
                    if j == tstart[b]:
                        agg = paggp.tile([128, 128], f32, tag="agg",
                                         name=f"agg_{i}_{b}")
                    nc.tensor.matmul(
                        agg[:], lhsT=st[:, j % SG, :], rhs=gt[:],
                        start=(j == tstart[b]), stop=(j == tend[b] - 1))
                    if j == tend[b] - 1:
                        s_sb = blkp.tile([128, 128], f32, tag="s_sb")
                        nc.vector.tensor_scalar(
                            out=s_sb[:], in0=agg[:],
                            scalar1=dinv09_sb[:, b:b + 1], scalar2=None,
                            op0=mybir.AluOpType.mult)
                        nc.vector.tensor_tensor(
                            out=s_sb[:], in0=s_sb[:],
                            in1=h0s_sb[:, b * 128:(b + 1) * 128],
                            op=mybir.AluOpType.add)
                        stp = ptpp.tile([128, 128], f32, tag="tp")
                        nc.tensor.transpose(out=stp[:], in_=s_sb[:],
                                            identity=ident_sb[:])
                        st_sb = blkp.tile([128, 128], f32, tag="st_sb")
                        nc.scalar.copy(out=st_sb[:], in_=stp[:])
                        z = pzp.tile([128, 128], f32, tag="z")
                        nc.tensor.matmul(z[:], lhsT=st_sb[:],
                                         rhs=wt_sb[:d, i * d:(i + 1) * d],
                                         start=True, stop=True)
                        if not last:
                            go = blkp.tile([128, 128], f16, tag="gout")
                            nc.scalar.activation(
                                out=go[:], in_=z[:],
                                func=mybir.ActivationFunctionType.Relu,
                                scale=dinvp_sb[:, b:b + 1])
                            nc.sync.dma_start(
                                out=ag_in_t[b * 128:(b + 1) * 128, :],
                                in_=go[:])
                        else:
                            h_sb = blkp.tile([128, 128], f32, tag="s_sb")
                            nc.scalar.activation(
                                out=h_sb[:], in_=z[:],
                                func=mybir.ActivationFunctionType.Relu)
                            htp = ptpp.tile([128, 128], f32, tag="tp")
                            nc.tensor.transpose(out=htp[:], in_=h_sb[:],
                                                identity=ident_sb[:])
                            ht_sb = blkp.tile([128, 128], f32, tag="st_sb")
                            nc.scalar.copy(out=ht_sb[:], in_=htp[:])
                            op = pzp.tile([128, 128], f32, tag="z")
                            nc.tensor.matmul(op[:, :1], lhsT=ht_sb[:d],
                                             rhs=wout_sb[:d], start=True,
                                             stop=True)
                            o_sb = blkp.tile([128, 1], f32, tag="o_sb")
                            nc.vector.tensor_scalar(
                                out=o_sb[:], in0=op[:, :1],
                                scalar1=bout_sb[:, :1], scalar2=None,
                                op0=mybir.AluOpType.add)
                            nv = min(128, npc - b * 128)
                            nc.sync.dma_start(
                                out=yout_t[b * 128:b * 128 + nv, :],
                                in_=o_sb[:nv])
                if not last:
                    nc.gpsimd.collective_compute(
                        "AllGather", mybir.AluOpType.bypass, replica_groups=rg,
                        ins=[ag_in_t[:]], outs=[dst_tab[:]])

    nc.finalize()
    return nc


def _get_program(meta):
    key = (meta["n"], meta["d"], meta["ncores"], meta["layers"],
           meta["tiles_b"])
    if key not in _BUILD_CACHE:
        _BUILD_CACHE[key] = _build_program(meta)
    return _BUILD_CACHE[key]


def _install_profile_shim():
    """Register the axon NTFF profiling hook if absent (for trace=True)."""
    import types
    if "antenv.axon_hooks" in sys.modules:
        return
    try:
        from trn_agent_boot.trn_boot import _ntff_profile_via_ctypes
        hook = _ntff_profile_via_ctypes("/opt/axon/libaxon_pjrt.so")
    except Exception:
        return
    mod = types.ModuleType("antenv.axon_hooks")
    mod.get_axon_ntff_profile_hook = lambda: hook
    mod.set_axon_ntff_profile_hook = lambda h: None
    sys.modules["antenv.axon_hooks"] = mod


def kernel(x, edge_weight, W_in, b_in, W_gcn, W_out, b_out, edge_index,
           trace=False):
    global LAST_RESULTS
    from concourse.bass_utils import run_bass_kernel_spmd

    x = np.asarray(x)
    in_maps, meta = _host_prepare(
        x, W_in, b_in, W_gcn, W_out, b_out, np.asarray(edge_index),
        N, NCORES, LAYERS, ALPHA, THETA)
    nc = _get_program(meta)
    if trace:
        _install_profile_shim()
    res = run_bass_kernel_spmd(nc, in_maps, core_ids=list(range(NCORES)),
                               trace=trace)
    LAST_RESULTS = res
    npc = meta["npc"]
    y = np.empty((N, 1), np.float32)
    for c in range(NCORES):
        y[c * npc:(c + 1) * npc] = res.results[c]["yout"][:npc]
    return y
